# revision 1
# baseline (speedup 1.0000x reference)
"""ConvGRU Trainium2 kernel.

video [B=2, T=16, C=128, H=64, W=64] f32; 1x1-conv GRU over T.
Sharding: data-parallel over (B x H/16) -> 8 cores, each core owns
P = 16*64 = 1024 pixels for all T; weights replicated.

Per core, per timestep (pixels on the free dim, channels on partitions):
    zr_pre = [Wzx@x + Wzh@h | Wrx@x + Wrh@h]      (PE, fp16 in / fp32 psum)
    z = sigmoid(zr_pre[:P] + bz); r = sigmoid(zr_pre[P:] + br)   (ACT)
    rh = r * h                                     (DVE)
    c = tanh(Whx@x + Whh@rh + bh)                  (PE + ACT)
    h = h + z * (c - h)                            (DVE, fp16 state)

The recurrence is latency-bound: each pixel group's step is a serial
cross-engine chain (h -> Wrh matmul -> sigmoid -> r*h -> Whh matmul ->
tanh -> blend -> h').  Structure choices below all serve that chain:
  - x-side matmul contributions for step t+1 are issued into step t's
    tail (PSUM c-tiles double buffered) to keep the PE dense/warm
  - r-gate work goes first (it gates the tanh matmul); z sigmoids are
    slotted late (only needed by the final blend)
  - group priority alternates per step so the second group's queueing
    penalty averages out instead of compounding on one chain
  - warmup matmuls + an early dummy activation hide the HAM clock-gate
    ramp and the ACT table load behind the initial x DMA

Numerics: fp16 matmul inputs/gates/state, fp32 PSUM accum + fp32 bias.
"""

import os
import sys

import numpy as np

B, T, C, H, W = 2, 16, 128, 64, 64
NCORES = 8
HQ = H // 4          # 16 rows of H per core (4 H-slices x 2 batches = 8 cores)
P = HQ * W           # 1024 pixels per core
G = 2                # pixel groups per step (independent recurrence chains)
PG = P // G          # 512 pixels per group

_PROG = None


def _ensure_paths():
    for p in ("/opt/trn_rl_repo",):
        if p not in sys.path and os.path.isdir(p):
            sys.path.append(p)


def _build():
    _ensure_paths()
    import concourse.bacc as bacc
    import concourse.tile as tile
    from concourse import mybir

    f32 = mybir.dt.float32
    f16 = mybir.dt.float16
    AF = mybir.ActivationFunctionType

    nc = bacc.Bacc(
        "TRN2", target_bir_lowering=False, debug=False, num_devices=NCORES
    )
    x_dram = nc.dram_tensor("x_seq", [T, C, P], f16, kind="ExternalInput")
    w_dram = nc.dram_tensor("wmats", [C, 6 * C], f16, kind="ExternalInput")
    b_dram = nc.dram_tensor("biases", [C, 4], f32, kind="ExternalInput")
    o_dram = nc.dram_tensor("out_seq", [T, C, P], f16, kind="ExternalOutput")

    x_ap = x_dram.ap()
    w_ap = w_dram.ap()
    b_ap = b_dram.ap()
    o_ap = o_dram.ap()

    WZX, WZH, WRX, WRH, WHX, WHH = range(6)

    with tile.TileContext(nc) as tc:
        with (
            tc.tile_pool(name="consts", bufs=1) as consts,
            tc.tile_pool(name="xin", bufs=4) as xpool,
            tc.tile_pool(name="state", bufs=2) as spool,
            tc.tile_pool(name="work", bufs=2) as wk,
            tc.tile_pool(name="ps", bufs=1, space="PSUM") as ps,
        ):
            wt = consts.tile([C, 6 * C], f16)
            nc.sync.dma_start(wt[:], w_ap[:])
            bt = consts.tile([C, 4], f32)
            nc.gpsimd.dma_start(bt[:], b_ap[:])

            def wslice(i):
                return wt[:, i * C : (i + 1) * C]

            # fp16 state per pixel group
            h16 = []
            for g in range(G):
                t16 = spool.tile([C, PG], f16, tag=f"h16_{g}")
                nc.vector.memset(t16[:], 0.0)
                h16.append(t16)

            # -- warmup: ramp the PE clock gate + preload the ACT table
            #    while the first x DMA is in flight --
            warm = ps.tile([C, PG], f32, tag="zr_0")
            for i in range(5):
                nc.tensor.matmul(
                    warm[:], wslice(i % 6), wt[:, :PG],
                    start=True, stop=True,
                )
            wtmp = wk.tile([C, PG], f16, tag="r_0")
            nc.scalar.activation(
                wtmp[:], warm[:], AF.Sigmoid, bias=bt[:, 0:1]
            )

            def load_x(t):
                xt = xpool.tile([C, P], f16, tag="x")
                nc.sync.dma_start(xt[:], x_ap[t])
                return xt

            def open_zr(xt, gorder):
                """Open z|r accumulations with the x-side contributions."""
                zr_t = [None] * G
                for g in gorder:
                    xs = xt[:, g * PG : (g + 1) * PG]
                    zr = ps.tile([C, 2 * PG], f32, tag=f"zr_{g}", bufs=1)
                    nc.tensor.matmul(
                        zr[:, PG:], wslice(WRX), xs, start=True, stop=False
                    )
                    nc.tensor.matmul(
                        zr[:, :PG], wslice(WZX), xs, start=True, stop=False
                    )
                    zr_t[g] = zr
                return zr_t

            def open_c(xt, gorder):
                cp_t = [None] * G
                for g in gorder:
                    xs = xt[:, g * PG : (g + 1) * PG]
                    cp = ps.tile([C, PG], f32, tag=f"c_{g}", bufs=2)
                    nc.tensor.matmul(
                        cp[:], wslice(WHX), xs, start=True, stop=False
                    )
                    cp_t[g] = cp
                return cp_t

            first = list(range(G))
            x_t = load_x(0)
            zr_t = open_zr(x_t, first)
            cp_t = open_c(x_t, first)

            for t in range(T):
                go = first if t % 2 == 0 else first[::-1]
                x_next = load_x(t + 1) if t + 1 < T else None

                # -- PE: close the r then z accumulations (chain head) --
                for g in go:
                    nc.tensor.matmul(
                        zr_t[g][:, PG:], wslice(WRH), h16[g][:],
                        start=False, stop=True,
                    )
                for g in go:
                    nc.tensor.matmul(
                        zr_t[g][:, :PG], wslice(WZH), h16[g][:],
                        start=False, stop=True,
                    )

                # -- ACT: r sigmoids first (they gate rh -> c matmul) --
                r16 = [None] * G
                for g in go:
                    rt = wk.tile([C, PG], f16, tag=f"r_{g}")
                    nc.scalar.activation(
                        rt[:], zr_t[g][:, PG:], AF.Sigmoid, bias=bt[:, 1:2]
                    )
                    r16[g] = rt

                rh16 = [None] * G
                for g in go:
                    rh = wk.tile([C, PG], f16, tag=f"rh_{g}")
                    nc.vector.tensor_mul(rh[:], r16[g][:], h16[g][:])
                    rh16[g] = rh

                for g in go:
                    nc.tensor.matmul(
                        cp_t[g][:], wslice(WHH), rh16[g][:],
                        start=False, stop=True,
                    )

                # next step's c openers can run any time (double-buffered)
                cp_next = open_c(x_next, go) if x_next is not None else None

                # -- ACT: zbar/tanh interleaved; zbar = 1-z = sigmoid(-pre)
                #    feeds the blend h' = zbar*h + (1-zbar)*c, whose only
                #    post-tanh serial ops are v = z*c and h' = u + v --
                zb16, c16 = [None] * G, [None] * G
                for g in go:
                    zbt = wk.tile([C, PG], f16, tag=f"zb_{g}")
                    nc.scalar.activation(
                        zbt[:], zr_t[g][:, :PG], AF.Sigmoid,
                        bias=bt[:, 3:4], scale=-1.0,
                    )
                    zb16[g] = zbt
                    ct = wk.tile([C, PG], f16, tag=f"c16_{g}")
                    nc.scalar.activation(
                        ct[:], cp_t[g][:], AF.Tanh, bias=bt[:, 2:3]
                    )
                    c16[g] = ct

                # next step's z|r openers (wait on this step's sigmoids)
                zr_next = open_zr(x_next, go) if x_next is not None else None

                # -- DVE mid-chain: u = zbar*h and z = 1-zbar overlap the
                #    tanh; only v and the final add trail it --
                u16, z16 = [None] * G, [None] * G
                for g in go:
                    ut = wk.tile([C, PG], f16, tag=f"u_{g}")
                    nc.vector.tensor_mul(ut[:], zb16[g][:], h16[g][:])
                    u16[g] = ut
                    zt = wk.tile([C, PG], f16, tag=f"z_{g}")
                    nc.vector.tensor_scalar(
                        zt[:], zb16[g][:], -1.0, 1.0,
                        mybir.AluOpType.mult, mybir.AluOpType.add,
                    )
                    z16[g] = zt

                for g in go:
                    v16 = wk.tile([C, PG], f16, tag=f"v_{g}")
                    nc.vector.tensor_mul(v16[:], z16[g][:], c16[g][:])
                    n16 = spool.tile([C, PG], f16, tag=f"h16_{g}")
                    nc.vector.tensor_add(n16[:], u16[g][:], v16[:])
                    h16[g] = n16
                    nc.sync.dma_start(
                        o_ap[t, :, g * PG : (g + 1) * PG], n16[:]
                    )

                if x_next is not None:
                    x_t, zr_t, cp_t = x_next, zr_next, cp_next

    nc.compile()
    return nc


def _get_prog():
    global _PROG
    if _PROG is None:
        _PROG = _build()
    return _PROG


def _make_in_maps(video, Wz, bz, Wr, br, Wh, bh):
    w6 = np.concatenate(
        [
            Wz[:, :C].T, Wz[:, C:].T,
            Wr[:, :C].T, Wr[:, C:].T,
            Wh[:, :C].T, Wh[:, C:].T,
        ],
        axis=1,
    ).astype(np.float16)
    b3 = np.stack([bz, br, bh, -bz], axis=1).astype(np.float32)
    in_maps = []
    for core in range(NCORES):
        b_, q = divmod(core, 4)
        xs = np.ascontiguousarray(
            video[b_, :, :, q * HQ : (q + 1) * HQ, :]
        ).reshape(T, C, P).astype(np.float16)
        in_maps.append({"x_seq": xs, "wmats": w6, "biases": b3})
    return in_maps


def kernel(video, Wz, bz, Wr, br, Wh, bh):
    _ensure_paths()
    from concourse.bass_utils import run_bass_kernel_spmd

    video = np.asarray(video, dtype=np.float32)
    nc = _get_prog()
    in_maps = _make_in_maps(video, Wz, bz, Wr, br, Wh, bh)
    res = run_bass_kernel_spmd(nc, in_maps, list(range(NCORES)))

    out = np.empty((B, T, C, H, W), np.float32)
    for core in range(NCORES):
        b_, q = divmod(core, 4)
        out[b_, :, :, q * HQ : (q + 1) * HQ, :] = np.asarray(
            res.results[core]["out_seq"]
        ).astype(np.float32).reshape(T, C, HQ, W)
    return out



# revision 3
# speedup vs baseline: 1.0151x; 1.0151x over previous
"""ConvGRU Trainium2 kernel.

video [B=2, T=16, C=128, H=64, W=64] f32; 1x1-conv GRU over T.
Sharding: data-parallel over (B x H/16) -> 8 cores, each core owns
P = 16*64 = 1024 pixels for all T; weights replicated.

Per core, per timestep (pixels on the free dim, channels on partitions),
two independent pixel groups (hi/lo) pipeline the serial recurrence:
    r  = sigmoid(Wrx@x + Wrh@h + br)        (PE -> ACT)
    zb = sigmoid(-(Wzx@x + Wzh@h + bz))     (PE -> ACT)   zb = 1-z
    rh = r * h                              (DVE)
    c  = tanh(Whx@x + Whh@rh + bh)          (PE -> ACT)
    z  = 1 - zb                             (DVE, off critical path)
    u  = zb * h                             (POOL, off critical path)
    v  = z * c ; h' = u + v                 (DVE, chain tail)

Engine balance: ACT runs the 6 sigmoids/tanh per step (~4.5us, the
bottleneck), DVE runs rh/z/v/h' (8 ops), Pool takes u (its only load),
PE runs 12 matmuls with the t+1 c-openers slotted into the rh-wait gap
to stay dense (clock ramp) and the t+1 zr-openers at the tail.

Numerics: fp16 matmul inputs/gates/state, fp32 PSUM accum + fp32 bias.
"""

import os
import sys

import numpy as np

B, T, C, H, W = 2, 16, 128, 64, 64
NCORES = 8
HQ = H // 4          # 16 rows of H per core (4 H-slices x 2 batches = 8 cores)
P = HQ * W           # 1024 pixels per core
G = 2                # pixel groups per step (independent recurrence chains)
PG = P // G          # 512 pixels per group

_PROG = None


def _ensure_paths():
    for p in ("/opt/trn_rl_repo",):
        if p not in sys.path and os.path.isdir(p):
            sys.path.append(p)


def _build():
    _ensure_paths()
    import concourse.bacc as bacc
    import concourse.tile as tile
    from concourse import mybir

    f32 = mybir.dt.float32
    f16 = mybir.dt.float16
    AF = mybir.ActivationFunctionType

    nc = bacc.Bacc(
        "TRN2", target_bir_lowering=False, debug=False, num_devices=NCORES
    )
    x_dram = nc.dram_tensor("x_seq", [T, C, P], f16, kind="ExternalInput")
    w_dram = nc.dram_tensor("wmats", [C, 6 * C], f16, kind="ExternalInput")
    b_dram = nc.dram_tensor("biases", [C, 4], f32, kind="ExternalInput")
    o_dram = nc.dram_tensor("out_seq", [T, C, P], f16, kind="ExternalOutput")

    x_ap = x_dram.ap()
    w_ap = w_dram.ap()
    b_ap = b_dram.ap()
    o_ap = o_dram.ap()

    WZX, WZH, WRX, WRH, WHX, WHH = range(6)
    BR, BH, NBZ = 0, 1, 2  # bias columns: br, bh, -bz

    with tile.TileContext(nc) as tc:
        with (
            tc.tile_pool(name="consts", bufs=1) as consts,
            tc.tile_pool(name="xin", bufs=4) as xpool,
            tc.tile_pool(name="state", bufs=2) as spool,
            tc.tile_pool(name="work", bufs=2) as wk,
            tc.tile_pool(name="ps", bufs=1, space="PSUM") as ps,
        ):
            wt = consts.tile([C, 6 * C], f16)
            nc.sync.dma_start(wt[:], w_ap[:])
            bt = consts.tile([C, 4], f32)
            nc.gpsimd.dma_start(bt[:], b_ap[:])

            def wslice(i):
                return wt[:, i * C : (i + 1) * C]

            # fp16 state per pixel group
            h16 = []
            for g in range(G):
                t16 = spool.tile([C, PG], f16, tag=f"h16_{g}")
                nc.vector.memset(t16[:], 0.0)
                h16.append(t16)

            # PSUM: z tiles (banks 0-1), r tiles (2-3), c tiles dbl-buf (4-7)
            zp = [
                ps.tile([C, PG], f32, tag=f"z_{g}", bufs=1, name=f"zp{g}")
                for g in range(G)
            ]
            rp = [
                ps.tile([C, PG], f32, tag=f"r_{g}", bufs=1, name=f"rp{g}")
                for g in range(G)
            ]

            # -- warmup: ramp the PE clock gate + preload the ACT table
            #    while the first x DMA is in flight --
            for i in range(5):
                nc.tensor.matmul(
                    zp[0][:], wslice(i % 6), wt[:, :PG],
                    start=True, stop=True,
                )
            wtmp = wk.tile([C, PG], f16, tag="r16_0")
            nc.scalar.activation(
                wtmp[:], zp[0][:], AF.Sigmoid, bias=bt[:, BR : BR + 1]
            )

            def load_x(t):
                xt = xpool.tile([C, P], f16, tag="x")
                nc.sync.dma_start(xt[:], x_ap[t])
                return xt

            def open_zr(xt, gorder):
                """Open z and r accumulations with the x-side contributions."""
                for g in gorder:
                    xs = xt[:, g * PG : (g + 1) * PG]
                    nc.tensor.matmul(
                        rp[g][:], wslice(WRX), xs, start=True, stop=False
                    )
                    nc.tensor.matmul(
                        zp[g][:], wslice(WZX), xs, start=True, stop=False
                    )

            def open_c(xt, gorder):
                cp_t = [None] * G
                for g in gorder:
                    xs = xt[:, g * PG : (g + 1) * PG]
                    cp = ps.tile([C, PG], f32, tag=f"c_{g}", bufs=2)
                    nc.tensor.matmul(
                        cp[:], wslice(WHX), xs, start=True, stop=False
                    )
                    cp_t[g] = cp
                return cp_t

            first = list(range(G))
            x_t = load_x(0)
            open_zr(x_t, first)
            cp_t = open_c(x_t, first)

            for t in range(T):
                go = first if t % 2 == 0 else first[::-1]
                x_next = load_x(t + 1) if t + 1 < T else None

                # -- PE: close r then z accumulations (chain head) --
                for g in go:
                    nc.tensor.matmul(
                        rp[g][:], wslice(WRH), h16[g][:],
                        start=False, stop=True,
                    )
                for g in go:
                    nc.tensor.matmul(
                        zp[g][:], wslice(WZH), h16[g][:],
                        start=False, stop=True,
                    )

                # -- ACT: r sigmoid, then zb sigmoid per group --
                r16, zb16 = [None] * G, [None] * G
                for g in go:
                    rt = wk.tile([C, PG], f16, tag=f"r16_{g}")
                    nc.scalar.activation(
                        rt[:], rp[g][:], AF.Sigmoid, bias=bt[:, BR : BR + 1]
                    )
                    r16[g] = rt
                    zbt = wk.tile([C, PG], f16, tag=f"zb_{g}")
                    nc.scalar.activation(
                        zbt[:], zp[g][:], AF.Sigmoid,
                        bias=bt[:, NBZ : NBZ + 1], scale=-1.0,
                    )
                    zb16[g] = zbt

                # -- DVE chain: rh gates the c matmul; z off-path --
                rh16, z16 = [None] * G, [None] * G
                for g in go:
                    rh = wk.tile([C, PG], f16, tag=f"rh_{g}")
                    nc.vector.tensor_mul(rh[:], r16[g][:], h16[g][:])
                    rh16[g] = rh
                    zt = wk.tile([C, PG], f16, tag=f"z_{g}")
                    nc.vector.tensor_scalar(
                        zt[:], zb16[g][:], -1.0, 1.0,
                        mybir.AluOpType.mult, mybir.AluOpType.add,
                    )
                    z16[g] = zt

                # -- POOL: u = zb*h, off the critical path --
                u16 = [None] * G
                for g in go:
                    ut = wk.tile([C, PG], f16, tag=f"u_{g}")
                    nc.gpsimd.tensor_mul(ut[:], zb16[g][:], h16[g][:])
                    u16[g] = ut

                # next step's c openers fill the rh-wait gap on the PE
                cp_next = open_c(x_next, go) if x_next is not None else None

                for g in go:
                    nc.tensor.matmul(
                        cp_t[g][:], wslice(WHH), rh16[g][:],
                        start=False, stop=True,
                    )

                # -- ACT: tanh per group --
                c16 = [None] * G
                for g in go:
                    ct = wk.tile([C, PG], f16, tag=f"c16_{g}")
                    nc.scalar.activation(
                        ct[:], cp_t[g][:], AF.Tanh, bias=bt[:, BH : BH + 1]
                    )
                    c16[g] = ct

                # next step's zr openers (wait on this step's sigmoids)
                if x_next is not None:
                    open_zr(x_next, go)

                # -- DVE chain tail: v = z*c ; h' = u + v --
                for g in go:
                    v16 = wk.tile([C, PG], f16, tag=f"v_{g}")
                    nc.vector.tensor_mul(v16[:], z16[g][:], c16[g][:])
                    n16 = spool.tile([C, PG], f16, tag=f"h16_{g}")
                    nc.vector.tensor_add(n16[:], u16[g][:], v16[:])
                    h16[g] = n16
                    nc.sync.dma_start(
                        o_ap[t, :, g * PG : (g + 1) * PG], n16[:]
                    )

                if x_next is not None:
                    x_t, cp_t = x_next, cp_next

    nc.compile()
    return nc


def _get_prog():
    global _PROG
    if _PROG is None:
        _PROG = _build()
    return _PROG


def _make_in_maps(video, Wz, bz, Wr, br, Wh, bh):
    w6 = np.concatenate(
        [
            Wz[:, :C].T, Wz[:, C:].T,
            Wr[:, :C].T, Wr[:, C:].T,
            Wh[:, :C].T, Wh[:, C:].T,
        ],
        axis=1,
    ).astype(np.float16)
    b3 = np.stack([br, bh, -bz, bz], axis=1).astype(np.float32)
    in_maps = []
    for core in range(NCORES):
        b_, q = divmod(core, 4)
        xs = np.ascontiguousarray(
            video[b_, :, :, q * HQ : (q + 1) * HQ, :]
        ).reshape(T, C, P).astype(np.float16)
        in_maps.append({"x_seq": xs, "wmats": w6, "biases": b3})
    return in_maps


def kernel(video, Wz, bz, Wr, br, Wh, bh):
    _ensure_paths()
    from concourse.bass_utils import run_bass_kernel_spmd

    video = np.asarray(video, dtype=np.float32)
    nc = _get_prog()
    in_maps = _make_in_maps(video, Wz, bz, Wr, br, Wh, bh)
    res = run_bass_kernel_spmd(nc, in_maps, list(range(NCORES)))

    out = np.empty((B, T, C, H, W), np.float32)
    for core in range(NCORES):
        b_, q = divmod(core, 4)
        out[b_, :, :, q * HQ : (q + 1) * HQ, :] = np.asarray(
            res.results[core]["out_seq"]
        ).astype(np.float32).reshape(T, C, HQ, W)
    return out
